# revision 1
# baseline (speedup 1.0000x reference)
"""Trainium2 Bass kernel for LocalSLC GNN message passing.

Computation (per batch b):
    y[b,n,o] = sum_{k,i} bs[n,k] * ws[k,i,o] * x[b, knn_ids[n,k], i]

Shapes: B=16, N=10000, K=16, C_IN=C_OUT=64, fp32.

Strategy (8 NeuronCores, data-parallel over batch, 2 batches/core):
  * Host packs x for core c as xpair[n, 0:64]=x[2c], xpair[n, 64:128]=x[2c+1],
    so one gathered 512B row serves both batches (halves gather traffic and
    hits the >=512B/descriptor DMA sweet spot).
  * Per 128-node tile: one multi-index indirect DMA gathers the 16 neighbor
    rows per node from DRAM into SBUF G[128, 16, 128].
  * DVE tensor_scalar (per-partition scalars = bs tile columns, 2x_2P mode)
    scales G by bs in place.
  * Per k: PE transposes G[:, k, :] tiles into [(2b,i), n] chunks (PSUM,
    one accumulation group per bank), ACT escapes PSUM -> SBUF rounding to
    float32r, then a full-rate f32r PE matmul with a stationary
    block-diagonal W2[k] = diag(ws[k], ws[k]) accumulates y[(2b,o), n]
    over the 16 k's in one PSUM bank.
  * y escapes via DVE and DMAs out as yT[b, o, n]; host transposes back.
"""

import numpy as np

import concourse.bass as bass
import concourse.tile as tile
from concourse import bacc, mybir
from concourse.masks import make_identity

B, N, K, CI, CO = 16, 10000, 16, 64, 64
NCORES = 8
BPC = B // NCORES  # 2 batches per core
NPAD = 10240  # pad N to a multiple of 512
TS = 128  # nodes per tile


def round_f32r(a):
    """Round fp32 array to the float32r grid (11-bit mantissa, RNE)."""
    u = a.astype(np.float32).view(np.uint32)
    low = u & 0xFFF
    add = (low > 0x800) | ((low == 0x800) & (((u >> 12) & 1) == 1))
    return (((u >> 12) + add.astype(np.uint32)) << 12).view(np.float32)


def build_program(npad=NPAD, sb_tiles=4):
    """Build the per-core Bass program (identical on all 8 cores)."""
    nt = npad // TS
    nsb = nt // sb_tiles
    assert nsb * sb_tiles == nt
    sbn = sb_tiles * TS  # nodes per superblock (one psum bank: <=512 fp32)
    assert sbn <= 512

    nc = bacc.Bacc("TRN2", target_bir_lowering=False, debug=False)
    f32, f32r, i32 = mybir.dt.float32, mybir.dt.float32r, mybir.dt.int32

    nt_ = npad // TS
    xpair = nc.dram_tensor("xpair", [npad, 2 * CI], f32, kind="ExternalInput").ap()
    # wrapped int16 gather indices: per (tile, k-half) a [128, 64] block in
    # dma_gather's "wrapped in 16 partitions, replicated across cores" layout
    idsw = nc.dram_tensor(
        "idsw", [nt_, 2, 128, 64], mybir.dt.int16, kind="ExternalInput"
    ).ap()
    bsd = nc.dram_tensor("bs", [npad, K], f32, kind="ExternalInput").ap()
    # block-diag W2[k] = [[ws[k], 0], [0, ws[k]]], host-rounded to f32r
    wts = nc.dram_tensor("w2", [K, 2 * CI, 2 * CO], f32r, kind="ExternalInput").ap()
    yT = nc.dram_tensor("yT", [BPC, CO, npad], f32, kind="ExternalOutput").ap()

    with tile.TileContext(nc) as tc:
        with (
            tc.tile_pool(name="const", bufs=1) as const_pool,
            tc.tile_pool(name="meta", bufs=2 * sb_tiles) as meta_pool,
            tc.tile_pool(name="g", bufs=2 * sb_tiles) as g_pool,
            tc.tile_pool(name="zts", bufs=3) as zts_pool,
            tc.tile_pool(name="ysb", bufs=2) as ysb_pool,
            tc.tile_pool(name="ztp", bufs=3, space="PSUM") as ztp_pool,
            tc.tile_pool(name="yp", bufs=2, space="PSUM") as yp_pool,
        ):
            ident = const_pool.tile([128, 128], f32)
            make_identity(nc, ident[:])
            w2_s = const_pool.tile([128, K, 2 * CO], f32r)
            for k in range(K):
                nc.sync.dma_start(out=w2_s[:, k, :], in_=wts[k])

            for sbi in range(nsb):
                t0 = sbi * sb_tiles
                # one batched DMA per superblock for indices and bs scalars
                ids_sb = meta_pool.tile(
                    [128, sb_tiles, 2, 64], mybir.dt.int16, tag="ids"
                )
                nc.sync.dma_start(
                    out=ids_sb[:],
                    in_=idsw[t0:t0 + sb_tiles].rearrange("t h p s -> p t h s"),
                )
                bs_sb = meta_pool.tile([TS, sb_tiles, K], f32, tag="bs")
                nc.sync.dma_start(
                    out=bs_sb[:],
                    in_=bsd[t0 * TS:(t0 + sb_tiles) * TS, :].rearrange(
                        "(t p) k -> p t k", p=TS
                    ),
                )
                g_tiles = []
                for t in range(sb_tiles):
                    g = g_pool.tile([TS, K, 2 * CI], f32, tag="g")
                    for h in range(2):
                        nc.gpsimd.dma_gather(
                            out_ap=g[:, h * (K // 2):(h + 1) * (K // 2), :],
                            in_ap=xpair[:],
                            idxs_ap=ids_sb[:, t, h, :],
                            num_idxs=1024,
                            num_idxs_reg=1024,
                            elem_size=2 * CI,
                        )
                    for k in range(K):
                        nc.vector.tensor_scalar_mul(
                            g[:, k, :], g[:, k, :], bs_sb[:, t, k:k + 1]
                        )
                    g_tiles.append(g)

                y_ps = yp_pool.tile([2 * CO, sbn], f32, tag="y")
                for k in range(K):
                    zt_ps = ztp_pool.tile([128, sbn], f32, tag="ztp")
                    for t in range(sb_tiles):
                        nc.tensor.matmul(
                            zt_ps[:, t * TS:(t + 1) * TS],
                            lhsT=g_tiles[t][:, k, :],
                            rhs=ident[:],
                            is_transpose=True,
                            start=(t == 0),
                            stop=(t == sb_tiles - 1),
                        )
                    zt_sb = zts_pool.tile([128, sbn], f32r, tag="zts")
                    nc.scalar.copy(out=zt_sb[:], in_=zt_ps[:])
                    nc.tensor.matmul(
                        y_ps[:],
                        lhsT=w2_s[:, k, :],
                        rhs=zt_sb[:],
                        start=(k == 0),
                        stop=(k == K - 1),
                    )
                y_sb = ysb_pool.tile([2 * CO, sbn], f32, tag="ysb")
                nc.vector.tensor_copy(out=y_sb[:], in_=y_ps[:])
                for b in range(BPC):
                    nc.sync.dma_start(
                        out=yT[b, :, sbi * sbn:(sbi + 1) * sbn],
                        in_=y_sb[b * CO:(b + 1) * CO, :],
                    )

    nc.compile()
    return nc


_CACHE = {}


def _get_program():
    if "nc" not in _CACHE:
        _CACHE["nc"] = build_program()
    return _CACHE["nc"]


def _wrap_ids(ids_p, npad=NPAD):
    """Build dma_gather wrapped-int16 index blocks [nt, 2, 128, 64].

    Per (tile, k-half): flat order j = k_local*128 + n_local (so gathered
    row j lands on partition j%128, free slot j//128 = k_local), then
    wrapped w[p, s] = flat[s*16 + p] and replicated across the 8 Q7 cores.
    """
    nt = npad // TS
    a = ids_p.reshape(nt, TS, 2, K // 2)      # [t, n, h, kl]
    a = a.transpose(0, 2, 3, 1)               # [t, h, kl, n] -> flat kl*128+n
    f = a.reshape(nt, 2, 1024)
    w = f.reshape(nt, 2, 64, 16).transpose(0, 1, 3, 2)  # [t, h, 16, 64]
    return np.ascontiguousarray(np.tile(w, (1, 1, 8, 1)).astype(np.int16))


def _pack_inputs(x, knn_ids, bs, ws):
    """Host-side packing into per-core input maps."""
    ids_p = np.zeros((NPAD, K), np.int32)
    ids_p[:N] = knn_ids
    idsw = _wrap_ids(ids_p)
    bs_p = np.zeros((NPAD, K), np.float32)
    bs_p[:N] = bs
    w2 = np.zeros((K, 2 * CI, 2 * CO), np.float32)
    w2[:, :CI, :CO] = ws
    w2[:, CI:, CO:] = ws
    w2 = round_f32r(w2)
    in_maps = []
    for c in range(NCORES):
        xp = np.zeros((NPAD, 2 * CI), np.float32)
        xp[:N, :CI] = x[2 * c]
        xp[:N, CI:] = x[2 * c + 1]
        in_maps.append({"xpair": xp, "idsw": idsw, "bs": bs_p, "w2": w2})
    return in_maps


def kernel(x, knn_ids, bs, ws):
    from concourse import bass_utils

    x = np.asarray(x, np.float32)
    knn_ids = np.asarray(knn_ids, np.int32)
    bs = np.asarray(bs, np.float32)
    ws = np.asarray(ws, np.float32)

    nc = _get_program()
    in_maps = _pack_inputs(x, knn_ids, bs, ws)
    try:
        res = bass_utils.run_bass_kernel_spmd(
            nc, in_maps, core_ids=list(range(NCORES))
        )
    except Exception:
        # one retry: a crashed previous tenant can leave a core in
        # NRT_EXEC_UNIT_UNRECOVERABLE until the next nrt_init resets it
        res = bass_utils.run_bass_kernel_spmd(
            nc, in_maps, core_ids=list(range(NCORES))
        )

    y = np.empty((B, N, CO), np.float32)
    for c in range(NCORES):
        yt = res.results[c]["yT"]  # [BPC, CO, NPAD]
        for b in range(BPC):
            y[BPC * c + b] = yt[b, :, :N].T
    return y



# revision 35
# speedup vs baseline: 1.8020x; 1.8020x over previous
"""Trainium2 Bass kernel for LocalSLC GNN message passing.

Computation (per batch b):
    y[b,n,o] = sum_{k,i} bs[n,k] * ws[k,i,o] * x[b, knn_ids[n,k], i]

Shapes: B=16, N=10000, K=16, C_IN=C_OUT=64, fp32 in/out; bf16 on-chip.

Strategy (8 NeuronCores, hybrid sharding: 2 node-halves x 4 batch-quads):
  * Core c = (half h, quad q) handles batches 4q..4q+3 for nodes
    [5120h, 5120h+5120). x is packed per quad as xq[n, (4b,64i)] bf16, so
    one gathered row is 512B (the DMA sweet spot) and carries 4 batches.
    bf16 halves the dominant gather traffic vs fp32 (41.9MB/core).
  * Per superblock (4 node-tiles = 512 nodes): one 8192-index dma_gather
    pulls G[128n, (4t,16k), (4b,64i)] from DRAM.
  * DVE tensor_scalar (per-partition scalars = bs, 4x_2p mode) scales
    G's (t,k)-slices in place.
  * Per (tile, batch): 8 PE transposes of k-PAIR windows g[:, 2k, b, :]
    -> PSUM bf16 zt[(k0 i, k1 i), n] (bf16 identity: 1 cyc/row), escaped
    PSUM->SBUF by DVE/ACT/Pool copies (round-robin, the 1x-rate step).
  * 8 accumulating matmuls: stationary = zt-slice [(2k,i), n], moving =
    dense Wpair[(2k,i), o] (no block-diagonal waste) -> y[n, (b,o)] PSUM.
  * y escapes to SBUF as bf16 and DMAs out; host reassembles/upcasts.
"""

import numpy as np
import ml_dtypes

import concourse.bass as bass
import concourse.tile as tile
from concourse import bacc, mybir
from concourse.masks import make_identity

B, N, K, CI, CO = 16, 10000, 16, 64, 64
NCORES = 8
QB = 4            # batches per core (batch quad)
NPAD = 10240      # padded total nodes (gather source range)
NHALF = NPAD // 2 # nodes per core (node half)
TS = 128          # nodes per tile
TPS = 2           # tiles per superblock
NSB = NHALF // (TPS * TS)  # superblocks per core (10)
KP = K // 2       # k-pairs

BF16 = ml_dtypes.bfloat16


def build_program():
    """Build the per-core Bass program (identical on all 8 cores)."""
    nc = bacc.Bacc("TRN2", target_bir_lowering=False, debug=False)
    f32, bf16, i16 = mybir.dt.float32, mybir.dt.bfloat16, mybir.dt.int16

    NU = NSB * TPS
    xq = nc.dram_tensor("xq", [NPAD, QB * CI], bf16, kind="ExternalInput").ap()
    # wrapped int16 gather indices: per (node-tile, k-half) [128, 64] in
    # dma_gather's "wrapped in 16 partitions, replicated across 8 Q7 cores"
    # layout. num_idxs is hardware-capped at 1024 per gather instruction.
    idsw = nc.dram_tensor("idsw", [NU, 2, 128, 64], i16, kind="ExternalInput").ap()
    # per-partition scalars for the bs scale: [S][p, t, k]
    bsw = nc.dram_tensor("bsw", [NSB, 128, TPS, K], f32, kind="ExternalInput").ap()
    # dense per-k-pair weights: wp[kp][(j,i), o] = ws[2kp+j, i, o]
    wp = nc.dram_tensor("wp", [KP, 2 * CI, CO], bf16, kind="ExternalInput").ap()
    yout = nc.dram_tensor(
        "yout", [NSB * TPS, TS, QB * CO], bf16, kind="ExternalOutput"
    ).ap()

    with tile.TileContext(nc) as tc:
        with (
            tc.tile_pool(name="const", bufs=1) as const_pool,
            tc.tile_pool(name="g", bufs=5) as g_pool,
            tc.tile_pool(name="g5", bufs=4) as g5_pool,
            tc.tile_pool(name="zts", bufs=6) as zts_pool,
            tc.tile_pool(name="ysb", bufs=2) as ysb_pool,
            tc.tile_pool(name="ztp", bufs=3, space="PSUM") as ztp_pool,
            tc.tile_pool(name="yp", bufs=2, space="PSUM") as yp_pool,
        ):
            ident = const_pool.tile([128, 128], bf16)
            make_identity(nc, ident[:])
            # preload all gather indices and bs scalars up front: removes
            # per-superblock DMA dependencies from the gather critical path.
            # Superblock 0's indices load first so gather(0) starts ASAP.
            ids_all = const_pool.tile([128, NU, 2, 64], i16)
            nc.sync.dma_start(
                out=ids_all[:, 0, :, :], in_=idsw[0].rearrange("h p c -> p h c")
            )
            nc.sync.dma_start(
                out=ids_all[:, 1:, :, :],
                in_=idsw[1:].rearrange("u h p c -> p u h c"),
            )
            bs_all = const_pool.tile([128, NSB, TPS, K], f32)
            nc.sync.dma_start(
                out=bs_all[:], in_=bsw[:].rearrange("s p t k -> p s t k")
            )
            wp_s = const_pool.tile([128, KP, CO], bf16)
            nc.sync.dma_start(
                out=wp_s[:], in_=wp[:].rearrange("k p o -> p k o")
            )

            # zt escapes split DVE/ACT only: Pool must stay clear for
            # gather descriptor gen (in-order Pool stream), it takes the
            # small y escapes instead.
            esc_engines = [
                nc.vector, nc.scalar, nc.vector, nc.scalar,
            ]

            g_tiles = {}

            def emit_gather(u):
                # two 1024-index gathers per node-tile (HW caps num_idxs at
                # 1024): each covers one k-half, landing in slots 8h..8h+7
                g4 = g_pool.tile([128, K, QB, CI], bf16, tag="g")
                for h in range(2):
                    nc.gpsimd.dma_gather(
                        out_ap=g4[:, 8 * h:8 * h + 8, :, :].rearrange(
                            "p s b i -> p s (b i)"
                        ),
                        in_ap=xq[:],
                        idxs_ap=ids_all[:, u, h, :],
                        num_idxs=8 * TS,
                        num_idxs_reg=8 * TS,
                        elem_size=QB * CI,
                    )
                g_tiles[u] = g4
            zt_sbs = {}     # unit -> list of 4 escaped zt tiles
            yps = {}        # unit -> y psum tile
            esc_i = 0

            g5_tiles = {}

            def emit_scale(u):
                # scale while scattering into the k-pair layout
                # g5[128, kp, b, j, i]: makes the k-pair transpose windows
                # contiguous (the matmul stationary port needs 1 free dim)
                S, t = divmod(u, TPS)
                g4 = g_tiles.pop(u)
                g5 = g5_pool.tile([128, KP, QB, 2, CI], bf16, tag="g5")
                g5_tiles[u] = g5
                for k in range(K):
                    nc.vector.tensor_scalar_mul(
                        g5[:, k // 2, :, k % 2, :], g4[:, k, :, :],
                        bs_all[:, S, t, k:k + 1]
                    )

            def emit_transpose(u):
                g5 = g5_tiles[u]
                zt_sbs[u] = []
                # k-pair windows g5[:, kp, b, :, :] are contiguous 128 cols;
                # zt partitions become (k0 i, k1 i) for a single batch
                for bp in range(QB // 2):
                    # 2-bank psum tile: [b-in-pair][kp] blocks, one escape op
                    zt_ps = ztp_pool.tile([128, 2, KP, TS], bf16, tag="ztp")
                    for j in range(2):
                        b = 2 * bp + j
                        for p in range(KP):
                            nc.tensor.matmul(
                                zt_ps[:, j, p, :],
                                lhsT=g5[:, p, b, :, :],
                                rhs=ident[:],
                                is_transpose=True,
                                start=(p == 0),
                                stop=(p == KP - 1),
                            )
                    zt_sbs[u].append(zt_ps)

            zt_escaped = {}

            def emit_escape(u):
                nonlocal esc_i
                zt_escaped[u] = []
                for zt_ps in zt_sbs.pop(u):
                    zt_sb = zts_pool.tile([128, 2, KP, TS], bf16, tag="zts")
                    eng = esc_engines[esc_i % len(esc_engines)]
                    if eng is nc.scalar:
                        eng.copy(out=zt_sb[:], in_=zt_ps[:])
                    else:
                        eng.tensor_copy(out=zt_sb[:], in_=zt_ps[:])
                    esc_i += 1
                    zt_escaped[u].append(zt_sb)

            def emit_matmul(u):
                yp_ps = yp_pool.tile([TS, QB, CO], f32, tag="yp")
                yps[u] = yp_ps
                for bp in range(QB // 2):
                    for j in range(2):
                        b = 2 * bp + j
                        for p in range(KP):
                            nc.tensor.matmul(
                                yp_ps[:, b, :],
                                lhsT=zt_escaped[u][bp][:, j, p, :],
                                rhs=wp_s[:, p, :],
                                start=(p == 0),
                                stop=(p == KP - 1),
                            )
                del zt_escaped[u]
                del g5_tiles[u]

            def emit_out(u):
                y_sb = ysb_pool.tile([TS, QB, CO], bf16, tag="ysb")
                nc.scalar.copy(out=y_sb[:], in_=yps.pop(u)[:])
                nc.sync.dma_start(out=yout[u], in_=y_sb[:])

            emit_gather(0)
            emit_gather(1)
            for u in range(NU + 4):
                if u + 2 < NU:
                    emit_gather(u + 2)
                if u < NU:
                    emit_scale(u)
                if 0 <= u - 1 < NU:
                    emit_transpose(u - 1)
                if 0 <= u - 2 < NU:
                    emit_escape(u - 2)
                if 0 <= u - 3 < NU:
                    emit_matmul(u - 3)
                if 0 <= u - 4 < NU:
                    emit_out(u - 4)

    nc.compile()
    return nc


_CACHE = {}


def _get_program():
    if "nc" not in _CACHE:
        _CACHE["nc"] = build_program()
    return _CACHE["nc"]


def _wrap_ids_half(ids_half):
    """Wrapped int16 gather indices for one node-half: [NU, 2, 128, 64].

    Per (node-tile u, k-half h): flat j = k_local*TS + p -> edge
    (node u*128+p, 8h+k_local). Wrapped w[pp, s] = flat[s*16 + pp],
    replicated across the 8 Q7 cores.
    """
    NU = NSB * TPS
    a = ids_half.reshape(NU, TS, 2, 8)          # [u, p, h, kl]
    a = a.transpose(0, 2, 3, 1)                 # [u, h, kl, p] -> flat j
    f = a.reshape(NU, 2, 8 * TS)
    w = f.reshape(NU, 2, 64, 16).transpose(0, 1, 3, 2)   # [u, h, 16, 64]
    return np.ascontiguousarray(np.tile(w, (1, 1, 8, 1)).astype(np.int16))


def _pack_inputs(x, knn_ids, bs, ws):
    """Host-side packing into per-core input maps."""
    ids_p = np.zeros((NPAD, K), np.int32)
    ids_p[:N] = knn_ids
    bs_p = np.zeros((NPAD, K), np.float32)
    bs_p[:N] = bs

    wp = np.ascontiguousarray(
        ws.reshape(KP, 2 * CI, CO).astype(BF16)
    )

    idsw_h, bsw_h = [], []
    for h in range(2):
        sl = slice(h * NHALF, (h + 1) * NHALF)
        idsw_h.append(_wrap_ids_half(ids_p[sl]))
        b4 = bs_p[sl].reshape(NSB, TPS, TS, K).transpose(0, 2, 1, 3)
        bsw_h.append(np.ascontiguousarray(b4))

    xq_q = []
    for q in range(QB):
        xqv = np.zeros((NPAD, QB, CI), np.float32)
        xqv[:N] = x[4 * q:4 * q + 4].transpose(1, 0, 2)
        xq_q.append(np.ascontiguousarray(
            xqv.reshape(NPAD, QB * CI).astype(BF16)
        ))

    in_maps = []
    for c in range(NCORES):
        h, q = c // 4, c % 4
        in_maps.append({
            "xq": xq_q[q], "idsw": idsw_h[h], "bsw": bsw_h[h], "wp": wp,
        })
    return in_maps


def _unpack_outputs(res):
    """Reassemble full [B, N, CO] fp32 output from per-core yout tensors."""
    y = np.empty((B, N, CO), np.float32)
    for c in range(NCORES):
        h, q = c // 4, c % 4
        yt = np.asarray(res.results[c]["yout"]).astype(np.float32)
        # yt: [NSB*TPS, TS, QB, CO] -> nodes h*NHALF + nt*TS + p
        yt = yt.reshape(NSB * TPS * TS, QB, CO)
        n0 = h * NHALF
        n1 = min(n0 + NHALF, N)
        if n1 <= n0:
            continue
        for j in range(QB):
            y[4 * q + j, n0:n1] = yt[: n1 - n0, j]
    return y


def kernel(x, knn_ids, bs, ws):
    from concourse import bass_utils

    x = np.asarray(x, np.float32)
    knn_ids = np.asarray(knn_ids, np.int32)
    bs = np.asarray(bs, np.float32)
    ws = np.asarray(ws, np.float32)

    nc = _get_program()
    in_maps = _pack_inputs(x, knn_ids, bs, ws)
    try:
        res = bass_utils.run_bass_kernel_spmd(
            nc, in_maps, core_ids=list(range(NCORES))
        )
    except Exception:
        # one retry: a crashed previous tenant can leave a core in
        # NRT_EXEC_UNIT_UNRECOVERABLE until the next nrt_init resets it
        res = bass_utils.run_bass_kernel_spmd(
            nc, in_maps, core_ids=list(range(NCORES))
        )
    return _unpack_outputs(res)


# revision 36
# speedup vs baseline: 1.8830x; 1.0449x over previous
"""Trainium2 Bass kernel for LocalSLC GNN message passing.

Computation (per batch b):
    y[b,n,o] = sum_{k,i} bs[n,k] * ws[k,i,o] * x[b, knn_ids[n,k], i]

Shapes: B=16, N=10000, K=16, C_IN=C_OUT=64, fp32 in/out; bf16 on-chip.

Strategy (8 NeuronCores, hybrid sharding: 2 node-halves x 4 batch-quads):
  * Core c = (half h, quad q) handles batches 4q..4q+3 for nodes
    [5120h, 5120h+5120). x is packed per quad as xq[n, (4b,64i)] bf16, so
    one gathered row is 512B (the DMA sweet spot) and carries 4 batches.
    bf16 halves the dominant gather traffic vs fp32 (41.9MB/core).
  * Per superblock (4 node-tiles = 512 nodes): one 8192-index dma_gather
    pulls G[128n, (4t,16k), (4b,64i)] from DRAM.
  * DVE tensor_scalar (per-partition scalars = bs, 4x_2p mode) scales
    G's (t,k)-slices in place.
  * Per (tile, batch): 8 PE transposes of k-PAIR windows g[:, 2k, b, :]
    -> PSUM bf16 zt[(k0 i, k1 i), n] (bf16 identity: 1 cyc/row), escaped
    PSUM->SBUF by DVE/ACT/Pool copies (round-robin, the 1x-rate step).
  * 8 accumulating matmuls: stationary = zt-slice [(2k,i), n], moving =
    dense Wpair[(2k,i), o] (no block-diagonal waste) -> y[n, (b,o)] PSUM.
  * y escapes to SBUF as bf16 and DMAs out; host reassembles/upcasts.
"""

import numpy as np
import ml_dtypes

import concourse.bass as bass
import concourse.tile as tile
from concourse import bacc, mybir
from concourse.masks import make_identity

B, N, K, CI, CO = 16, 10000, 16, 64, 64
NCORES = 8
QB = 4            # batches per core (batch quad)
NPAD = 10240      # padded total nodes (gather source range)
NHALF = NPAD // 2 # nodes per core (node half)
TS = 128          # nodes per tile
TPS = 2           # tiles per superblock
NSB = NHALF // (TPS * TS)  # superblocks per core (10)
KP = K // 2       # k-pairs

BF16 = ml_dtypes.bfloat16


def build_program():
    """Build the per-core Bass program (identical on all 8 cores)."""
    nc = bacc.Bacc("TRN2", target_bir_lowering=False, debug=False)
    f32, bf16, i16 = mybir.dt.float32, mybir.dt.bfloat16, mybir.dt.int16

    NU = NSB * TPS
    xq = nc.dram_tensor("xq", [NPAD, QB * CI], bf16, kind="ExternalInput").ap()
    # wrapped int16 gather indices: per (node-tile, k-half) [128, 64] in
    # dma_gather's "wrapped in 16 partitions, replicated across 8 Q7 cores"
    # layout. num_idxs is hardware-capped at 1024 per gather instruction.
    idsw = nc.dram_tensor("idsw", [NU, 2, 128, 64], i16, kind="ExternalInput").ap()
    # per-partition scalars for the bs scale: [S][p, t, k]
    bsw = nc.dram_tensor("bsw", [NSB, 128, TPS, K], f32, kind="ExternalInput").ap()
    # dense per-k-pair weights: wp[kp][(j,i), o] = ws[2kp+j, i, o]
    wp = nc.dram_tensor("wp", [KP, 2 * CI, CO], bf16, kind="ExternalInput").ap()
    yout = nc.dram_tensor(
        "yout", [NSB * TPS, TS, QB * CO], bf16, kind="ExternalOutput"
    ).ap()

    with tile.TileContext(nc) as tc:
        with (
            tc.tile_pool(name="const", bufs=1) as const_pool,
            tc.tile_pool(name="g", bufs=3) as g_pool,
            tc.tile_pool(name="g5", bufs=4) as g5_pool,
            tc.tile_pool(name="zts", bufs=6) as zts_pool,
            tc.tile_pool(name="ysb", bufs=2) as ysb_pool,
            tc.tile_pool(name="ztp", bufs=3, space="PSUM") as ztp_pool,
            tc.tile_pool(name="yp", bufs=2, space="PSUM") as yp_pool,
        ):
            ident = const_pool.tile([128, 128], bf16)
            make_identity(nc, ident[:])
            # preload all gather indices and bs scalars up front: removes
            # per-superblock DMA dependencies from the gather critical path.
            # Superblock 0's indices load first so gather(0) starts ASAP.
            ids_all = const_pool.tile([128, NU, 2, 64], i16)
            nc.sync.dma_start(
                out=ids_all[:, 0, :, :], in_=idsw[0].rearrange("h p c -> p h c")
            )
            nc.sync.dma_start(
                out=ids_all[:, 1:, :, :],
                in_=idsw[1:].rearrange("u h p c -> p u h c"),
            )
            bs_all = const_pool.tile([128, NSB, TPS, K], f32)
            nc.sync.dma_start(
                out=bs_all[:], in_=bsw[:].rearrange("s p t k -> p s t k")
            )
            wp_s = const_pool.tile([128, KP, CO], bf16)
            nc.sync.dma_start(
                out=wp_s[:], in_=wp[:].rearrange("k p o -> p k o")
            )

            # zt escapes split DVE/ACT only: Pool must stay clear for
            # gather descriptor gen (in-order Pool stream), it takes the
            # small y escapes instead.
            esc_engines = [
                nc.vector, nc.scalar, nc.vector, nc.scalar,
            ]

            g_tiles = {}

            def emit_gather(u):
                # two 1024-index gathers per node-tile (HW caps num_idxs at
                # 1024): each covers one k-half, landing in slots 8h..8h+7
                g4 = g_pool.tile([128, K, QB, CI], bf16, tag="g")
                for h in range(2):
                    nc.gpsimd.dma_gather(
                        out_ap=g4[:, 8 * h:8 * h + 8, :, :].rearrange(
                            "p s b i -> p s (b i)"
                        ),
                        in_ap=xq[:],
                        idxs_ap=ids_all[:, u, h, :],
                        num_idxs=8 * TS,
                        num_idxs_reg=8 * TS,
                        elem_size=QB * CI,
                    )
                g_tiles[u] = g4
            zt_sbs = {}     # unit -> list of 4 escaped zt tiles
            yps = {}        # unit -> y psum tile
            esc_i = 0

            g5_tiles = {}

            def emit_scale(u):
                # scale while scattering into the k-pair layout
                # g5[128, kp, b, j, i]: makes the k-pair transpose windows
                # contiguous (the matmul stationary port needs 1 free dim)
                S, t = divmod(u, TPS)
                g4 = g_tiles.pop(u)
                g5 = g5_pool.tile([128, KP, QB, 2, CI], bf16, tag="g5")
                g5_tiles[u] = g5
                for k in range(K):
                    nc.vector.tensor_scalar_mul(
                        g5[:, k // 2, :, k % 2, :], g4[:, k, :, :],
                        bs_all[:, S, t, k:k + 1]
                    )

            def emit_transpose(u):
                g5 = g5_tiles[u]
                zt_sbs[u] = []
                # k-pair windows g5[:, kp, b, :, :] are contiguous 128 cols;
                # zt partitions become (k0 i, k1 i) for a single batch
                for bp in range(QB // 2):
                    # 2-bank psum tile: [b-in-pair][kp] blocks, one escape op
                    zt_ps = ztp_pool.tile([128, 2, KP, TS], bf16, tag="ztp")
                    for j in range(2):
                        b = 2 * bp + j
                        for p in range(KP):
                            nc.tensor.matmul(
                                zt_ps[:, j, p, :],
                                lhsT=g5[:, p, b, :, :],
                                rhs=ident[:],
                                is_transpose=True,
                                start=(p == 0),
                                stop=(p == KP - 1),
                            )
                    zt_sbs[u].append(zt_ps)

            zt_escaped = {}

            def emit_escape(u):
                nonlocal esc_i
                zt_escaped[u] = []
                for zt_ps in zt_sbs.pop(u):
                    zt_sb = zts_pool.tile([128, 2, KP, TS], bf16, tag="zts")
                    eng = esc_engines[esc_i % len(esc_engines)]
                    if eng is nc.scalar:
                        eng.copy(out=zt_sb[:], in_=zt_ps[:])
                    else:
                        eng.tensor_copy(out=zt_sb[:], in_=zt_ps[:])
                    esc_i += 1
                    zt_escaped[u].append(zt_sb)

            def emit_matmul(u):
                yp_ps = yp_pool.tile([TS, QB, CO], f32, tag="yp")
                yps[u] = yp_ps
                for bp in range(QB // 2):
                    for j in range(2):
                        b = 2 * bp + j
                        for p in range(KP):
                            nc.tensor.matmul(
                                yp_ps[:, b, :],
                                lhsT=zt_escaped[u][bp][:, j, p, :],
                                rhs=wp_s[:, p, :],
                                start=(p == 0),
                                stop=(p == KP - 1),
                            )
                del zt_escaped[u]
                del g5_tiles[u]

            def emit_out(u):
                y_sb = ysb_pool.tile([TS, QB, CO], bf16, tag="ysb")
                nc.scalar.copy(out=y_sb[:], in_=yps.pop(u)[:])
                nc.sync.dma_start(out=yout[u], in_=y_sb[:])

            emit_gather(0)
            emit_gather(1)
            for u in range(NU + 4):
                if u + 2 < NU:
                    emit_gather(u + 2)
                if u < NU:
                    emit_scale(u)
                if 0 <= u - 1 < NU:
                    emit_transpose(u - 1)
                if 0 <= u - 2 < NU:
                    emit_escape(u - 2)
                if 0 <= u - 3 < NU:
                    emit_matmul(u - 3)
                if 0 <= u - 4 < NU:
                    emit_out(u - 4)

    nc.compile()
    return nc


_CACHE = {}


def _get_program():
    if "nc" not in _CACHE:
        _CACHE["nc"] = build_program()
    return _CACHE["nc"]


def _wrap_ids_half(ids_half):
    """Wrapped int16 gather indices for one node-half: [NU, 2, 128, 64].

    Per (node-tile u, k-half h): flat j = k_local*TS + p -> edge
    (node u*128+p, 8h+k_local). Wrapped w[pp, s] = flat[s*16 + pp],
    replicated across the 8 Q7 cores.
    """
    NU = NSB * TPS
    a = ids_half.reshape(NU, TS, 2, 8)          # [u, p, h, kl]
    a = a.transpose(0, 2, 3, 1)                 # [u, h, kl, p] -> flat j
    f = a.reshape(NU, 2, 8 * TS)
    w = f.reshape(NU, 2, 64, 16).transpose(0, 1, 3, 2)   # [u, h, 16, 64]
    return np.ascontiguousarray(np.tile(w, (1, 1, 8, 1)).astype(np.int16))


def _pack_inputs(x, knn_ids, bs, ws):
    """Host-side packing into per-core input maps."""
    ids_p = np.zeros((NPAD, K), np.int32)
    ids_p[:N] = knn_ids
    bs_p = np.zeros((NPAD, K), np.float32)
    bs_p[:N] = bs

    wp = np.ascontiguousarray(
        ws.reshape(KP, 2 * CI, CO).astype(BF16)
    )

    idsw_h, bsw_h = [], []
    for h in range(2):
        sl = slice(h * NHALF, (h + 1) * NHALF)
        idsw_h.append(_wrap_ids_half(ids_p[sl]))
        b4 = bs_p[sl].reshape(NSB, TPS, TS, K).transpose(0, 2, 1, 3)
        bsw_h.append(np.ascontiguousarray(b4))

    xq_q = []
    for q in range(QB):
        xqv = np.zeros((NPAD, QB, CI), np.float32)
        xqv[:N] = x[4 * q:4 * q + 4].transpose(1, 0, 2)
        xq_q.append(np.ascontiguousarray(
            xqv.reshape(NPAD, QB * CI).astype(BF16)
        ))

    in_maps = []
    for c in range(NCORES):
        h, q = c // 4, c % 4
        in_maps.append({
            "xq": xq_q[q], "idsw": idsw_h[h], "bsw": bsw_h[h], "wp": wp,
        })
    return in_maps


def _unpack_outputs(res):
    """Reassemble full [B, N, CO] fp32 output from per-core yout tensors."""
    y = np.empty((B, N, CO), np.float32)
    for c in range(NCORES):
        h, q = c // 4, c % 4
        yt = np.asarray(res.results[c]["yout"]).astype(np.float32)
        # yt: [NSB*TPS, TS, QB, CO] -> nodes h*NHALF + nt*TS + p
        yt = yt.reshape(NSB * TPS * TS, QB, CO)
        n0 = h * NHALF
        n1 = min(n0 + NHALF, N)
        if n1 <= n0:
            continue
        for j in range(QB):
            y[4 * q + j, n0:n1] = yt[: n1 - n0, j]
    return y


def kernel(x, knn_ids, bs, ws):
    from concourse import bass_utils

    x = np.asarray(x, np.float32)
    knn_ids = np.asarray(knn_ids, np.int32)
    bs = np.asarray(bs, np.float32)
    ws = np.asarray(ws, np.float32)

    nc = _get_program()
    in_maps = _pack_inputs(x, knn_ids, bs, ws)
    try:
        res = bass_utils.run_bass_kernel_spmd(
            nc, in_maps, core_ids=list(range(NCORES))
        )
    except Exception:
        # one retry: a crashed previous tenant can leave a core in
        # NRT_EXEC_UNIT_UNRECOVERABLE until the next nrt_init resets it
        res = bass_utils.run_bass_kernel_spmd(
            nc, in_maps, core_ids=list(range(NCORES))
        )
    return _unpack_outputs(res)


# revision 38
# speedup vs baseline: 1.9190x; 1.0191x over previous
"""Trainium2 Bass kernel for LocalSLC GNN message passing.

Computation (per batch b):
    y[b,n,o] = sum_{k,i} bs[n,k] * ws[k,i,o] * x[b, knn_ids[n,k], i]

Shapes: B=16, N=10000, K=16, C_IN=C_OUT=64, fp32 in/out; bf16 on-chip.

Strategy (8 NeuronCores, hybrid sharding: 2 node-halves x 4 batch-quads):
  * Core c = (half h, quad q) handles batches 4q..4q+3 for nodes
    [5120h, 5120h+5120). x is packed per quad as xq[n, (4b,64i)] bf16, so
    one gathered row is 512B (the DMA sweet spot) and carries 4 batches.
    bf16 halves the dominant gather traffic vs fp32 (41.9MB/core).
  * Per 128-node tile: two 1024-index dma_gathers (HW caps num_idxs at
    1024) pull G[128n, 16k, (4b,64i)] from DRAM. Indices/bs/weights are
    all preloaded up front so nothing interrupts the gather stream.
  * DVE tensor_scalar (per-partition scalars = bs, 4x mode) scales G
    while scattering into a k-pair layout g5[128, kp, b, (2k,64i)] whose
    transpose windows are contiguous (matmul stationary APs must
    optimize to one free dimension).
  * PE transposes each [128n, (2k,1b,64i)] window -> PSUM bf16
    zt[(k0 i, k1 i), n] (bf16 identity: 1 cyc/row); DVE/ACT copies
    escape PSUM->SBUF (the 1x-rate step; Pool must stay clear so gather
    descriptor gen is never blocked in its in-order stream).
  * 8 accumulating 64-row matmuls per (tile, batch): stationary =
    zt-slice [(2k,i), n], moving = dense Wpair[(2k,i), o] (full
    128-contraction, no block-diagonal waste) -> y[n, (b,o)] PSUM.
  * y escapes to SBUF as bf16 and DMAs out; host reassembles/upcasts.
  * Emission is software-pipelined per tile-unit u with explicit lags
    (gather u+2, scale u, transpose u-1, escape u-2, matmul u-3,
    out u-4) so every instruction is data-ready when its in-order
    engine queue reaches it.

Modeled (TimelineSim) and measured: 156.6us vs 294.9us baseline.
"""

import numpy as np
import ml_dtypes

import concourse.bass as bass
import concourse.tile as tile
from concourse import bacc, mybir
from concourse.masks import make_identity

B, N, K, CI, CO = 16, 10000, 16, 64, 64
NCORES = 8
QB = 4            # batches per core (batch quad)
NPAD = 10240      # padded total nodes (gather source range)
NHALF = NPAD // 2 # nodes per core (node half)
TS = 128          # nodes per tile
TPS = 2           # tiles per superblock
NSB = NHALF // (TPS * TS)  # superblocks per core (10)
KP = K // 2       # k-pairs

BF16 = ml_dtypes.bfloat16


def build_program():
    """Build the per-core Bass program (identical on all 8 cores)."""
    nc = bacc.Bacc("TRN2", target_bir_lowering=False, debug=False)
    f32, bf16, i16 = mybir.dt.float32, mybir.dt.bfloat16, mybir.dt.int16

    NU = NSB * TPS
    xq = nc.dram_tensor("xq", [NPAD, QB * CI], bf16, kind="ExternalInput").ap()
    # wrapped int16 gather indices: per (node-tile, k-half) [128, 64] in
    # dma_gather's "wrapped in 16 partitions, replicated across 8 Q7 cores"
    # layout. num_idxs is hardware-capped at 1024 per gather instruction.
    idsw = nc.dram_tensor("idsw", [NU, 2, 128, 64], i16, kind="ExternalInput").ap()
    # per-partition scalars for the bs scale: [S][p, t, k]
    bsw = nc.dram_tensor("bsw", [NSB, 128, TPS, K], f32, kind="ExternalInput").ap()
    # dense per-k-pair weights: wp[kp][(j,i), o] = ws[2kp+j, i, o]
    wp = nc.dram_tensor("wp", [KP, 2 * CI, CO], bf16, kind="ExternalInput").ap()
    yout = nc.dram_tensor(
        "yout", [NSB * TPS, TS, QB * CO], bf16, kind="ExternalOutput"
    ).ap()

    with tile.TileContext(nc) as tc:
        with (
            tc.tile_pool(name="const", bufs=1) as const_pool,
            tc.tile_pool(name="g", bufs=3) as g_pool,
            tc.tile_pool(name="g5", bufs=4) as g5_pool,
            tc.tile_pool(name="zts", bufs=6) as zts_pool,
            tc.tile_pool(name="ysb", bufs=2) as ysb_pool,
            tc.tile_pool(name="ztp", bufs=3, space="PSUM") as ztp_pool,
            tc.tile_pool(name="yp", bufs=2, space="PSUM") as yp_pool,
        ):
            ident = const_pool.tile([128, 128], bf16)
            make_identity(nc, ident[:])
            # preload all gather indices and bs scalars up front: removes
            # per-superblock DMA dependencies from the gather critical path.
            # Superblock 0's indices load first so gather(0) starts ASAP.
            ids_all = const_pool.tile([128, NU, 2, 64], i16)
            nc.sync.dma_start(
                out=ids_all[:, 0:3, :, :],
                in_=idsw[0:3].rearrange("u h p c -> p u h c"),
            )
            nc.sync.dma_start(
                out=ids_all[:, 3:, :, :],
                in_=idsw[3:].rearrange("u h p c -> p u h c"),
            )
            bs_all = const_pool.tile([128, NSB, TPS, K], f32)
            nc.sync.dma_start(
                out=bs_all[:], in_=bsw[:].rearrange("s p t k -> p s t k")
            )
            wp_s = const_pool.tile([128, KP, CO], bf16)
            nc.sync.dma_start(
                out=wp_s[:], in_=wp[:].rearrange("k p o -> p k o")
            )

            # zt escapes split DVE/ACT only: Pool must stay clear for
            # gather descriptor gen (in-order Pool stream), it takes the
            # small y escapes instead.
            esc_engines = [
                nc.vector, nc.scalar, nc.vector, nc.scalar,
            ]

            g_tiles = {}

            def emit_gather(u):
                # two 1024-index gathers per node-tile (HW caps num_idxs at
                # 1024): each covers one k-half, landing in slots 8h..8h+7
                g4 = g_pool.tile([128, K, QB, CI], bf16, tag="g")
                for h in range(2):
                    nc.gpsimd.dma_gather(
                        out_ap=g4[:, 8 * h:8 * h + 8, :, :].rearrange(
                            "p s b i -> p s (b i)"
                        ),
                        in_ap=xq[:],
                        idxs_ap=ids_all[:, u, h, :],
                        num_idxs=8 * TS,
                        num_idxs_reg=8 * TS,
                        elem_size=QB * CI,
                    )
                g_tiles[u] = g4
            zt_sbs = {}     # unit -> list of 4 escaped zt tiles
            yps = {}        # unit -> y psum tile
            esc_i = 0

            g5_tiles = {}

            def emit_scale(u):
                # scale while scattering into the k-pair layout
                # g5[128, kp, b, j, i]: makes the k-pair transpose windows
                # contiguous (the matmul stationary port needs 1 free dim)
                S, t = divmod(u, TPS)
                g4 = g_tiles.pop(u)
                g5 = g5_pool.tile([128, KP, QB, 2, CI], bf16, tag="g5")
                g5_tiles[u] = g5
                for k in range(K):
                    nc.vector.tensor_scalar_mul(
                        g5[:, k // 2, :, k % 2, :], g4[:, k, :, :],
                        bs_all[:, S, t, k:k + 1]
                    )

            def emit_transpose(u):
                g5 = g5_tiles[u]
                zt_sbs[u] = []
                # k-pair windows g5[:, kp, b, :, :] are contiguous 128 cols;
                # zt partitions become (k0 i, k1 i) for a single batch
                for bp in range(QB // 2):
                    # 2-bank psum tile: [b-in-pair][kp] blocks, one escape op
                    zt_ps = ztp_pool.tile([128, 2, KP, TS], bf16, tag="ztp")
                    for j in range(2):
                        b = 2 * bp + j
                        for p in range(KP):
                            nc.tensor.matmul(
                                zt_ps[:, j, p, :],
                                lhsT=g5[:, p, b, :, :],
                                rhs=ident[:],
                                is_transpose=True,
                                start=(p == 0),
                                stop=(p == KP - 1),
                            )
                    zt_sbs[u].append(zt_ps)

            zt_escaped = {}

            def emit_escape(u):
                nonlocal esc_i
                zt_escaped[u] = []
                for zt_ps in zt_sbs.pop(u):
                    zt_sb = zts_pool.tile([128, 2, KP, TS], bf16, tag="zts")
                    eng = esc_engines[esc_i % len(esc_engines)]
                    if eng is nc.scalar:
                        eng.copy(out=zt_sb[:], in_=zt_ps[:])
                    else:
                        eng.tensor_copy(out=zt_sb[:], in_=zt_ps[:])
                    esc_i += 1
                    zt_escaped[u].append(zt_sb)

            def emit_matmul(u):
                yp_ps = yp_pool.tile([TS, QB, CO], f32, tag="yp")
                yps[u] = yp_ps
                for bp in range(QB // 2):
                    for j in range(2):
                        b = 2 * bp + j
                        for p in range(KP):
                            nc.tensor.matmul(
                                yp_ps[:, b, :],
                                lhsT=zt_escaped[u][bp][:, j, p, :],
                                rhs=wp_s[:, p, :],
                                start=(p == 0),
                                stop=(p == KP - 1),
                            )
                del zt_escaped[u]
                del g5_tiles[u]

            def emit_out(u):
                y_sb = ysb_pool.tile([TS, QB, CO], bf16, tag="ysb")
                nc.scalar.copy(out=y_sb[:], in_=yps.pop(u)[:])
                nc.sync.dma_start(out=yout[u], in_=y_sb[:])

            emit_gather(0)
            emit_gather(1)
            for u in range(NU + 4):
                if u + 2 < NU:
                    emit_gather(u + 2)
                if u < NU:
                    emit_scale(u)
                if 0 <= u - 1 < NU:
                    emit_transpose(u - 1)
                if 0 <= u - 2 < NU:
                    emit_escape(u - 2)
                if 0 <= u - 3 < NU:
                    emit_matmul(u - 3)
                if 0 <= u - 4 < NU:
                    emit_out(u - 4)

    nc.compile()
    return nc


_CACHE = {}


def _get_program():
    if "nc" not in _CACHE:
        _CACHE["nc"] = build_program()
    return _CACHE["nc"]


def _wrap_ids_half(ids_half):
    """Wrapped int16 gather indices for one node-half: [NU, 2, 128, 64].

    Per (node-tile u, k-half h): flat j = k_local*TS + p -> edge
    (node u*128+p, 8h+k_local). Wrapped w[pp, s] = flat[s*16 + pp],
    replicated across the 8 Q7 cores.
    """
    NU = NSB * TPS
    a = ids_half.reshape(NU, TS, 2, 8)          # [u, p, h, kl]
    a = a.transpose(0, 2, 3, 1)                 # [u, h, kl, p] -> flat j
    f = a.reshape(NU, 2, 8 * TS)
    w = f.reshape(NU, 2, 64, 16).transpose(0, 1, 3, 2)   # [u, h, 16, 64]
    return np.ascontiguousarray(np.tile(w, (1, 1, 8, 1)).astype(np.int16))


def _pack_inputs(x, knn_ids, bs, ws):
    """Host-side packing into per-core input maps."""
    ids_p = np.zeros((NPAD, K), np.int32)
    ids_p[:N] = knn_ids
    bs_p = np.zeros((NPAD, K), np.float32)
    bs_p[:N] = bs

    wp = np.ascontiguousarray(
        ws.reshape(KP, 2 * CI, CO).astype(BF16)
    )

    idsw_h, bsw_h = [], []
    for h in range(2):
        sl = slice(h * NHALF, (h + 1) * NHALF)
        idsw_h.append(_wrap_ids_half(ids_p[sl]))
        b4 = bs_p[sl].reshape(NSB, TPS, TS, K).transpose(0, 2, 1, 3)
        bsw_h.append(np.ascontiguousarray(b4))

    xq_q = []
    for q in range(QB):
        xqv = np.zeros((NPAD, QB, CI), np.float32)
        xqv[:N] = x[4 * q:4 * q + 4].transpose(1, 0, 2)
        xq_q.append(np.ascontiguousarray(
            xqv.reshape(NPAD, QB * CI).astype(BF16)
        ))

    in_maps = []
    for c in range(NCORES):
        h, q = c // 4, c % 4
        in_maps.append({
            "xq": xq_q[q], "idsw": idsw_h[h], "bsw": bsw_h[h], "wp": wp,
        })
    return in_maps


def _unpack_outputs(res):
    """Reassemble full [B, N, CO] fp32 output from per-core yout tensors."""
    y = np.empty((B, N, CO), np.float32)
    for c in range(NCORES):
        h, q = c // 4, c % 4
        yt = np.asarray(res.results[c]["yout"]).astype(np.float32)
        # yt: [NSB*TPS, TS, QB, CO] -> nodes h*NHALF + nt*TS + p
        yt = yt.reshape(NSB * TPS * TS, QB, CO)
        n0 = h * NHALF
        n1 = min(n0 + NHALF, N)
        if n1 <= n0:
            continue
        for j in range(QB):
            y[4 * q + j, n0:n1] = yt[: n1 - n0, j]
    return y


def kernel(x, knn_ids, bs, ws):
    from concourse import bass_utils

    x = np.asarray(x, np.float32)
    knn_ids = np.asarray(knn_ids, np.int32)
    bs = np.asarray(bs, np.float32)
    ws = np.asarray(ws, np.float32)

    nc = _get_program()
    in_maps = _pack_inputs(x, knn_ids, bs, ws)
    try:
        res = bass_utils.run_bass_kernel_spmd(
            nc, in_maps, core_ids=list(range(NCORES))
        )
    except Exception:
        # one retry: a crashed previous tenant can leave a core in
        # NRT_EXEC_UNIT_UNRECOVERABLE until the next nrt_init resets it
        res = bass_utils.run_bass_kernel_spmd(
            nc, in_maps, core_ids=list(range(NCORES))
        )
    return _unpack_outputs(res)
